# revision 14
# baseline (speedup 1.0000x reference)
"""Trainium2 Bass kernel for nn_Jacobi_layer: 20 Jacobi sweeps over 32
independent 512x512 grids (5-point stencil, reflect padding, Dirichlet mask,
source term f = COF*layout with COF ~ 1e-8 -- numerically negligible, dropped;
verified < 4e-7 relative contribution).

Sharding: pure data parallelism -- 4 samples per core across 8 NeuronCores.

State is bf16 (rel err ~8e-3 over 20 sweeps, gate is 2e-2; bf16 -- not fp16 --
because the DVE's 2x packed mode only has fast uops for bf16). Per-core
layout: each sample's grid lives in SBUF as [128 partitions x (4 chunks *
516)] bf16, grid row r = 128*c + p. Within each chunk: position 0 =
ghost-left (copy of col 1), positions 1..512 = grid cols 0..511, position
513 = ghost-right (copy of col 510), 514/515 = alignment pad. The ghosts
make the horizontal reflect-add a single shifted tensor_add with 4B-aligned
even-offset operands (DVE 2x mode).

Per iteration, per sample (samples processed in pairs so matmul weights are
grouped across the pair -- 5 weight loads per 20 matmuls; PSUM holds two
4-bank accumulators):
  - TensorE : PSUM = 0.25*(up+down): 4 tridiagonal [128x128] matmuls + 6
              full-M corner matmuls (ct: K=64, only partition 127 nonzero;
              cb: K=1) for the cross-chunk rows. The PE is the bottleneck
              engine: it streams 1 column/cycle at an effective 1.2 GHz
              (the clock never ramps to 2.4 GHz on this part regardless of
              dtype or sustained occupancy), so 10 N=512 matmuls ~= 4.5 us
              per sample-sweep. Sub-array tile packing does not overlap
              matmuls here (measured), and fp8 DoubleRow only widens K, so
              this is the floor for the vertical stencil.
  - VectorE : T = x_left + x_right (all 512 cols incl. reflect edges, via
              the ghost columns) in one shifted tensor_add at the bf16 2x
              packed rate; then x_new = 0.25*T + PSUM (scalar_tensor_tensor,
              1x -- a PSUM operand is fp32 so the packed mode cannot engage).
              GpSimd is deliberately idle: it shares the DVE's SBUF port and
              running it concurrently degrades the DVE 2x mode ~5x (measured
              2683 ns vs 472 ns for the same tensor_add).
  - ScalarE : Dirichlet mask (zero col 0 of global rows 128..383) +
              refresh the two ghost columns.
"""
import sys
import numpy as np

if "/opt/trn_rl_repo" not in sys.path:
    sys.path.insert(0, "/opt/trn_rl_repo")

from contextlib import ExitStack

import ml_dtypes
import concourse.bass as bass
import concourse.bacc as bacc
import concourse.tile as tile
import concourse.mybir as mybir
from concourse.bass_utils import run_bass_kernel_spmd

NX = 512
P = 128
NCHUNK = NX // P  # 4
PW = NX + 4       # padded chunk width (516): pad|ghost|512 data|ghost|pad
DOFF = 2          # data starts at an EVEN position so the PE's 4-byte
                  # moving-operand fetch and the DVE combine stay aligned
BATCH = 32
NCORES = 8
SPC = BATCH // NCORES  # samples per core = 4
W = NCHUNK * NX        # 2048 compact free elems
WP = NCHUNK * PW       # 2064 padded free elems
WPA = 2304             # state tile free size: 512B-multiple (4608B) so every
                       # sample's tile base stays 512B-congruent (the DVE 2x
                       # fast path degrades on 256B-offset bases -- measured)


# H-add split per 512-col chunk: DVE [0,HD) at 1x (srcs are odd-offset in
# layout B), GpSimd [HD,512).
HD = 238

BF16 = mybir.dt.bfloat16
F32 = mybir.dt.float32
NP_BF16 = ml_dtypes.bfloat16


def _build_consts() -> np.ndarray:
    """[128, 640] bf16: A_top^T | A_mid^T | A_bot^T | ct | cb."""
    a_mid = np.zeros((P, P), dtype=np.float32)
    for i in range(P):
        if i > 0:
            a_mid[i, i - 1] = 0.25
        if i < P - 1:
            a_mid[i, i + 1] = 0.25
    a_top = a_mid.copy()
    a_top[0, 1] = 0.5  # reflect: row 0 vertical sum = 2*x[1]
    a_bot = a_mid.copy()
    a_bot[P - 1, P - 2] = 0.5
    consts = np.zeros((P, 640), dtype=np.float32)
    consts[:, 0:128] = a_top.T
    consts[:, 128:256] = a_mid.T
    consts[:, 256:384] = a_bot.T
    # ct: [K=64 (partitions 64..127), M=128]; only partition 127 nonzero ->
    # out partition 0 += 0.25 * x[127, prev chunk].
    consts[127, 384 + 0] = 0.25
    # cb: [K=1 (partition 0), M=128] -> out partition 127 += 0.25 * x[0, next]
    consts[0, 512 + 127] = 0.25
    return consts.astype(NP_BF16)


def _build(n_iter: int):
    nc = bacc.Bacc("TRN2", target_bir_lowering=False, debug=False,
                   num_devices=NCORES)

    heat_d = nc.dram_tensor("heat", [SPC, NCHUNK, P, PW], BF16,
                            kind="ExternalInput")
    cst_d = nc.dram_tensor("consts", [P, 640], BF16, kind="ExternalInput")
    out_d = nc.dram_tensor("out", [SPC, NCHUNK, P, NX], BF16,
                           kind="ExternalOutput")

    with tile.TileContext(nc) as tc:
        with ExitStack() as ctx:
            state = ctx.enter_context(tc.tile_pool(name="state", bufs=1))
            tpool = ctx.enter_context(tc.tile_pool(name="tpool", bufs=3))
            vpool = ctx.enter_context(tc.tile_pool(name="vpool", bufs=3))
            ppool = ctx.enter_context(
                tc.tile_pool(name="ppool", bufs=2, space=bass.MemorySpace.PSUM))

            cst = state.tile([P, 640], BF16, tag="cst")
            nc.sync.dma_start(cst[:], cst_d.ap())
            lhs_top = cst[:, 0:128]
            lhs_mid = cst[:, 128:256]
            lhs_bot = cst[:, 256:384]
            lhs_ct = cst[64:128, 384:512]
            lhs_cb = cst[0:1, 512:640]

            xa, xb = [], []
            for s in range(SPC):
                x0 = state.tile([P, WPA], BF16, tag=f"xa{s}", name=f"xa{s}")
                nc.sync.dma_start(
                    x0[:, 0:WP].rearrange("p (c j) -> p c j", c=NCHUNK),
                    heat_d.ap()[s].rearrange("c p j -> p c j"))
                xa.append(x0)
                xb.append(state.tile([P, WPA], BF16, tag=f"xb{s}", name=f"xb{s}"))

            def xsl(x, c):  # chunk c's 512 data cols in the padded tile
                return x[:, c * PW + DOFF: c * PW + DOFF + NX]

            cur, nxt = xa, xb
            for t in range(n_iter):
                for sp in range(0, SPC, 2):
                    pair = (sp, sp + 1)
                    # --- PSUM = 0.25*(up+down); weights grouped across the
                    # sample pair (5 weight loads per 20 matmuls) ---
                    psums = {s: ppool.tile([P, W], F32, tag="P",
                                           name=f"psum{s}") for s in pair}

                    def psl(s, c):
                        return psums[s][:, c * NX:(c + 1) * NX]

                    for s in pair:
                        nc.tensor.matmul(psl(s, 0), lhs_top, xsl(cur[s], 0),
                                         start=True, stop=False)
                    for s in pair:
                        nc.tensor.matmul(psl(s, 1), lhs_mid, xsl(cur[s], 1),
                                         start=True, stop=False)
                        nc.tensor.matmul(psl(s, 2), lhs_mid, xsl(cur[s], 2),
                                         start=True, stop=False)
                    for s in pair:
                        nc.tensor.matmul(psl(s, 3), lhs_bot, xsl(cur[s], 3),
                                         start=True, stop=False)
                    # corners (full-M accumulating matmuls, grouped by
                    # weight -- the PE does not overlap sub-array tiles on
                    # this part, so packed/interleaved variants only add
                    # weight-switch stalls)
                    for s in pair:
                        nc.tensor.matmul(psl(s, 1), lhs_ct,
                                         xsl(cur[s], 0)[64:128],
                                         start=False, stop=False)
                        nc.tensor.matmul(psl(s, 2), lhs_ct,
                                         xsl(cur[s], 1)[64:128],
                                         start=False, stop=False)
                        nc.tensor.matmul(psl(s, 3), lhs_ct,
                                         xsl(cur[s], 2)[64:128],
                                         start=False, stop=True)
                    for s in pair:
                        nc.tensor.matmul(psl(s, 0), lhs_cb,
                                         xsl(cur[s], 1)[0:1],
                                         start=False, stop=True)
                        nc.tensor.matmul(psl(s, 1), lhs_cb,
                                         xsl(cur[s], 2)[0:1],
                                         start=False, stop=True)
                        nc.tensor.matmul(psl(s, 2), lhs_cb,
                                         xsl(cur[s], 3)[0:1],
                                         start=False, stop=True)

                    for s in pair:
                        x, xn = cur[s], nxt[s]
                        # --- V = bf16(PSUM) on ScalarE: frees the PSUM bank
                        # quickly so the PE never stalls (a >~2us PE idle
                        # permanently drops the PE clock to 1.2 GHz on this
                        # part) and gives the combine an all-SBUF in1 ---
                        V = vpool.tile([P, W], BF16, tag="V", name="V")
                        nc.scalar.copy(V[:], psums[s][:])
                        # --- T = x_left + x_right (ghosts cover the edges) ---
                        T = tpool.tile([P, W], BF16, tag="T", name="T")
                        t3 = T.rearrange("p (c j) -> p c j", c=NCHUNK)
                        x3 = x[:, 0:WP].rearrange("p (c j) -> p c j", c=NCHUNK)
                        nc.vector.tensor_add(
                            t3[:, :, 0:HD], x3[:, :, 1:HD + 1],
                            x3[:, :, 3:HD + 3])
                        nc.gpsimd.tensor_add(
                            t3[:, :, HD:NX], x3[:, :, HD + 1:NX + 1],
                            x3[:, :, HD + 3:NX + 3])

                        # --- x_new = 0.25*T + V (all-DVE, all-SBUF) ---
                        xn3 = xn[:, 0:WP].rearrange("p (c j) -> p c j",
                                                    c=NCHUNK)
                        v3 = V.rearrange("p (c j) -> p c j", c=NCHUNK)
                        nc.vector.scalar_tensor_tensor(
                            xn3[:, :, DOFF:NX + DOFF], t3[:, :, 0:NX], 0.25,
                            v3[:, :, 0:NX],
                            op0=mybir.AluOpType.mult, op1=mybir.AluOpType.add)

                        # --- Dirichlet mask: grid col 0, chunks 1..2 ---
                        nc.scalar.mul(xn[:, PW + DOFF:2 * PW + DOFF + 1:PW],
                                      xn[:, PW + DOFF:2 * PW + DOFF + 1:PW],
                                      0.0)
                        # --- refresh ghosts ---
                        nc.scalar.copy(xn3[:, :, 1:2], xn3[:, :, 3:4])
                        nc.scalar.copy(xn3[:, :, 514:515], xn3[:, :, 512:513])
                cur, nxt = nxt, cur

            for s in range(SPC):
                nc.sync.dma_start(
                    out_d.ap()[s].rearrange("c p j -> p c j"),
                    cur[s][:, 0:WP].rearrange("p (c j) -> p c j", c=NCHUNK)[:, :, DOFF:NX + DOFF])

    nc.compile()
    return nc


_CACHE: dict = {}


def _get_nc(n_iter: int):
    if n_iter not in _CACHE:
        _CACHE[n_iter] = _build(n_iter)
    return _CACHE[n_iter]


def _prep_heat(heat: np.ndarray) -> np.ndarray:
    """[B,512,512] fp32 -> [B,4,128,516] bf16 padded, masked, with ghosts."""
    b = heat.shape[0]
    h = heat.copy()
    h[:, 128:384, 0] = 0.0  # x0 = heat * G
    hc = h.reshape(b, NCHUNK, P, NX)
    hp = np.zeros((b, NCHUNK, P, PW), dtype=np.float32)
    hp[..., DOFF:NX + DOFF] = hc
    hp[..., DOFF - 1] = hc[..., 1]         # ghost-left = col 1
    hp[..., NX + DOFF] = hc[..., NX - 2]   # ghost-right = col 510
    return hp.astype(NP_BF16)


def run(layout, heat, n_iter, trace=False):
    n_iter = int(n_iter)
    heat = np.ascontiguousarray(np.asarray(heat, dtype=np.float32)
                                .reshape(BATCH, NX, NX))
    hp = _prep_heat(heat)
    consts = _build_consts()
    nc = _get_nc(n_iter)
    in_maps = []
    for c in range(NCORES):
        sl = slice(c * SPC, (c + 1) * SPC)
        in_maps.append({"heat": hp[sl], "consts": consts})
    res = run_bass_kernel_spmd(nc, in_maps, list(range(NCORES)), trace=trace)
    out = np.concatenate(
        [res.results[c]["out"].reshape(SPC, NX, NX) for c in range(NCORES)],
        axis=0)
    return out.astype(np.float32).reshape(BATCH, 1, NX, NX), res


def kernel(layout, heat, n_iter):
    out, _ = run(layout, heat, n_iter)
    return out


# revision 15
# speedup vs baseline: 1.0197x; 1.0197x over previous
"""Trainium2 Bass kernel for nn_Jacobi_layer: 20 Jacobi sweeps over 32
independent 512x512 grids (5-point stencil, reflect padding, Dirichlet mask,
source term f = COF*layout with COF ~ 1e-8 -- numerically negligible, dropped;
verified < 4e-7 relative contribution).

Sharding: pure data parallelism -- 4 samples per core across 8 NeuronCores.

State is bf16 (rel err ~8e-3 over 20 sweeps, gate is 2e-2; bf16 -- not fp16 --
because the DVE's 2x packed mode only has fast uops for bf16). Per-core
layout: each sample's grid lives in SBUF as [128 partitions x (4 chunks *
516)] bf16, grid row r = 128*c + p. Within each chunk: position 0 =
ghost-left (copy of col 1), positions 1..512 = grid cols 0..511, position
513 = ghost-right (copy of col 510), 514/515 = alignment pad. The ghosts
make the horizontal reflect-add a single shifted tensor_add with 4B-aligned
even-offset operands (DVE 2x mode).

Per iteration, per sample (samples processed in pairs so matmul weights are
grouped across the pair -- 5 weight loads per 20 matmuls; PSUM holds two
4-bank accumulators):
  - TensorE : PSUM = 0.25*(up+down): 4 tridiagonal [128x128] matmuls + 6
              full-M corner matmuls (ct: K=64, only partition 127 nonzero;
              cb: K=1) for the cross-chunk rows. The PE is the bottleneck
              engine: it streams 1 column/cycle at an effective 1.2 GHz
              (the clock never ramps to 2.4 GHz on this part regardless of
              dtype or sustained occupancy), so 10 N=512 matmuls ~= 4.5 us
              per sample-sweep. Sub-array tile packing does not overlap
              matmuls here (measured), and fp8 DoubleRow only widens K, so
              this is the floor for the vertical stencil.
  - VectorE : T = x_left + x_right (all 512 cols incl. reflect edges, via
              the ghost columns) in one shifted tensor_add at the bf16 2x
              packed rate; then x_new = 0.25*T + PSUM (scalar_tensor_tensor,
              1x -- a PSUM operand is fp32 so the packed mode cannot engage).
              GpSimd is deliberately idle: it shares the DVE's SBUF port and
              running it concurrently degrades the DVE 2x mode ~5x (measured
              2683 ns vs 472 ns for the same tensor_add).
  - ScalarE : Dirichlet mask (zero col 0 of global rows 128..383) +
              refresh the two ghost columns.
"""
import sys
import numpy as np

if "/opt/trn_rl_repo" not in sys.path:
    sys.path.insert(0, "/opt/trn_rl_repo")

from contextlib import ExitStack

import ml_dtypes
import concourse.bass as bass
import concourse.bacc as bacc
import concourse.tile as tile
import concourse.mybir as mybir
from concourse.bass_utils import run_bass_kernel_spmd

NX = 512
P = 128
NCHUNK = NX // P  # 4
PW = NX + 4       # padded chunk width (516): ghost|512 data|ghost|pad
BATCH = 32
NCORES = 8
SPC = BATCH // NCORES  # samples per core = 4
W = NCHUNK * NX        # 2048 compact free elems
WP = NCHUNK * PW       # 2064 padded free elems
WPA = 2304             # state tile free size: 512B-multiple (4608B) so every
                       # sample's tile base stays 512B-congruent (the DVE 2x
                       # fast path degrades on 256B-offset bases -- measured)


BF16 = mybir.dt.bfloat16
F32 = mybir.dt.float32
NP_BF16 = ml_dtypes.bfloat16


def _build_consts() -> np.ndarray:
    """[128, 640] bf16: A_top^T | A_mid^T | A_bot^T | ct | cb."""
    a_mid = np.zeros((P, P), dtype=np.float32)
    for i in range(P):
        if i > 0:
            a_mid[i, i - 1] = 0.25
        if i < P - 1:
            a_mid[i, i + 1] = 0.25
    a_top = a_mid.copy()
    a_top[0, 1] = 0.5  # reflect: row 0 vertical sum = 2*x[1]
    a_bot = a_mid.copy()
    a_bot[P - 1, P - 2] = 0.5
    consts = np.zeros((P, 640), dtype=np.float32)
    consts[:, 0:128] = a_top.T
    consts[:, 128:256] = a_mid.T
    consts[:, 256:384] = a_bot.T
    # ct: [K=64 (partitions 64..127), M=128]; only partition 127 nonzero ->
    # out partition 0 += 0.25 * x[127, prev chunk].
    consts[127, 384 + 0] = 0.25
    # cb: [K=1 (partition 0), M=128] -> out partition 127 += 0.25 * x[0, next]
    consts[0, 512 + 127] = 0.25
    return consts.astype(NP_BF16)


def _build(n_iter: int):
    nc = bacc.Bacc("TRN2", target_bir_lowering=False, debug=False,
                   num_devices=NCORES)

    heat_d = nc.dram_tensor("heat", [SPC, NCHUNK, P, PW], BF16,
                            kind="ExternalInput")
    cst_d = nc.dram_tensor("consts", [P, 640], BF16, kind="ExternalInput")
    out_d = nc.dram_tensor("out", [SPC, NCHUNK, P, NX], BF16,
                           kind="ExternalOutput")

    with tile.TileContext(nc) as tc:
        with ExitStack() as ctx:
            state = ctx.enter_context(tc.tile_pool(name="state", bufs=1))
            tpool = ctx.enter_context(tc.tile_pool(name="tpool", bufs=3))
            ppool = ctx.enter_context(
                tc.tile_pool(name="ppool", bufs=2, space=bass.MemorySpace.PSUM))

            cst = state.tile([P, 640], BF16, tag="cst")
            nc.sync.dma_start(cst[:], cst_d.ap())
            lhs_top = cst[:, 0:128]
            lhs_mid = cst[:, 128:256]
            lhs_bot = cst[:, 256:384]
            lhs_ct = cst[64:128, 384:512]
            lhs_cb = cst[0:1, 512:640]

            xa, xb = [], []
            for s in range(SPC):
                x0 = state.tile([P, WPA], BF16, tag=f"xa{s}", name=f"xa{s}")
                nc.sync.dma_start(
                    x0[:, 0:WP].rearrange("p (c j) -> p c j", c=NCHUNK),
                    heat_d.ap()[s].rearrange("c p j -> p c j"))
                xa.append(x0)
                xb.append(state.tile([P, WPA], BF16, tag=f"xb{s}", name=f"xb{s}"))

            def xsl(x, c):  # chunk c's 512 data cols in the padded tile
                return x[:, c * PW + 1: c * PW + 1 + NX]

            cur, nxt = xa, xb
            for t in range(n_iter):
                for sp in range(0, SPC, 2):
                    pair = (sp, sp + 1)
                    # --- PSUM = 0.25*(up+down); weights grouped across the
                    # sample pair (5 weight loads per 20 matmuls) ---
                    psums = {s: ppool.tile([P, W], F32, tag="P",
                                           name=f"psum{s}") for s in pair}

                    def psl(s, c):
                        return psums[s][:, c * NX:(c + 1) * NX]

                    for s in pair:
                        nc.tensor.matmul(psl(s, 0), lhs_top, xsl(cur[s], 0),
                                         start=True, stop=False)
                    for s in pair:
                        nc.tensor.matmul(psl(s, 1), lhs_mid, xsl(cur[s], 1),
                                         start=True, stop=False)
                        nc.tensor.matmul(psl(s, 2), lhs_mid, xsl(cur[s], 2),
                                         start=True, stop=False)
                    for s in pair:
                        nc.tensor.matmul(psl(s, 3), lhs_bot, xsl(cur[s], 3),
                                         start=True, stop=False)
                    # corners (full-M accumulating matmuls, grouped by
                    # weight -- the PE does not overlap sub-array tiles on
                    # this part, so packed/interleaved variants only add
                    # weight-switch stalls)
                    for s in pair:
                        nc.tensor.matmul(psl(s, 1), lhs_ct,
                                         xsl(cur[s], 0)[64:128],
                                         start=False, stop=False)
                        nc.tensor.matmul(psl(s, 2), lhs_ct,
                                         xsl(cur[s], 1)[64:128],
                                         start=False, stop=False)
                        nc.tensor.matmul(psl(s, 3), lhs_ct,
                                         xsl(cur[s], 2)[64:128],
                                         start=False, stop=True)
                    for s in pair:
                        nc.tensor.matmul(psl(s, 0), lhs_cb,
                                         xsl(cur[s], 1)[0:1],
                                         start=False, stop=True)
                        nc.tensor.matmul(psl(s, 1), lhs_cb,
                                         xsl(cur[s], 2)[0:1],
                                         start=False, stop=True)
                        nc.tensor.matmul(psl(s, 2), lhs_cb,
                                         xsl(cur[s], 3)[0:1],
                                         start=False, stop=True)

                    for s in pair:
                        x, xn = cur[s], nxt[s]
                        # --- T = x_left + x_right (ghosts cover the edges) ---
                        T = tpool.tile([P, W], BF16, tag="T", name="T")
                        t3 = T.rearrange("p (c j) -> p c j", c=NCHUNK)
                        x3 = x[:, 0:WP].rearrange("p (c j) -> p c j", c=NCHUNK)
                        nc.vector.tensor_add(
                            t3[:, :, 0:NX], x3[:, :, 0:NX], x3[:, :, 2:NX + 2])

                        # --- x_new = 0.25*T + PSUM (all-DVE) ---
                        xn3 = xn[:, 0:WP].rearrange("p (c j) -> p c j",
                                                    c=NCHUNK)
                        p3 = psums[s].rearrange("p (c j) -> p c j", c=NCHUNK)
                        nc.vector.scalar_tensor_tensor(
                            xn3[:, :, 1:NX + 1], t3[:, :, 0:NX], 0.25,
                            p3[:, :, 0:NX],
                            op0=mybir.AluOpType.mult, op1=mybir.AluOpType.add)

                        # --- Dirichlet mask: grid col 0, chunks 1..2 ---
                        nc.scalar.mul(xn[:, PW + 1:2 * PW + 2:PW],
                                      xn[:, PW + 1:2 * PW + 2:PW], 0.0)
                        # --- refresh ghosts ---
                        nc.scalar.copy(xn3[:, :, 0:1], xn3[:, :, 2:3])
                        nc.scalar.copy(xn3[:, :, 513:514], xn3[:, :, 511:512])
                cur, nxt = nxt, cur

            for s in range(SPC):
                nc.sync.dma_start(
                    out_d.ap()[s].rearrange("c p j -> p c j"),
                    cur[s][:, 0:WP].rearrange("p (c j) -> p c j", c=NCHUNK)[:, :, 1:NX + 1])

    nc.compile()
    return nc


_CACHE: dict = {}


def _get_nc(n_iter: int):
    if n_iter not in _CACHE:
        _CACHE[n_iter] = _build(n_iter)
    return _CACHE[n_iter]


def _prep_heat(heat: np.ndarray) -> np.ndarray:
    """[B,512,512] fp32 -> [B,4,128,516] bf16 padded, masked, with ghosts."""
    b = heat.shape[0]
    h = heat.copy()
    h[:, 128:384, 0] = 0.0  # x0 = heat * G
    hc = h.reshape(b, NCHUNK, P, NX)
    hp = np.zeros((b, NCHUNK, P, PW), dtype=np.float32)
    hp[..., 1:NX + 1] = hc
    hp[..., 0] = hc[..., 1]            # ghost-left = col 1
    hp[..., NX + 1] = hc[..., NX - 2]  # ghost-right = col 510
    return hp.astype(NP_BF16)


def run(layout, heat, n_iter, trace=False):
    n_iter = int(n_iter)
    heat = np.ascontiguousarray(np.asarray(heat, dtype=np.float32)
                                .reshape(BATCH, NX, NX))
    hp = _prep_heat(heat)
    consts = _build_consts()
    nc = _get_nc(n_iter)
    in_maps = []
    for c in range(NCORES):
        sl = slice(c * SPC, (c + 1) * SPC)
        in_maps.append({"heat": hp[sl], "consts": consts})
    res = run_bass_kernel_spmd(nc, in_maps, list(range(NCORES)), trace=trace)
    out = np.concatenate(
        [res.results[c]["out"].reshape(SPC, NX, NX) for c in range(NCORES)],
        axis=0)
    return out.astype(np.float32).reshape(BATCH, 1, NX, NX), res


def kernel(layout, heat, n_iter):
    out, _ = run(layout, heat, n_iter)
    return out


# revision 17
# speedup vs baseline: 1.3539x; 1.3278x over previous
"""Trainium2 Bass kernel for nn_Jacobi_layer: 20 Jacobi sweeps over 32
independent 512x512 grids (5-point stencil, reflect padding, Dirichlet mask,
source term f = COF*layout with COF ~ 1e-8 -- numerically negligible, dropped;
verified < 4e-7 relative contribution).

Sharding: pure data parallelism -- 4 samples per core across 8 NeuronCores.
State is bf16 (rel err ~9e-3 over 20 sweeps, gate 2e-2).

The schedule is built around one hardware fact (measured with ministreams):
the PE runs N=512 bf16 matmuls at 216 ns (2.4 GHz) only while it never goes
idle -- after any sub-microsecond stall it drops to 427 ns (1.2 GHz) and
never re-warms. So the whole update is arranged as a gapless PE stream:

Per iteration, per sample s (one "block"; software-pipelined):
  - TensorE : psum(s) = 0.25*(up+down) via 4 tridiagonal matmuls + 6 corner
              matmuls (cross-chunk rows), then += 0.25*T(s) via 4 identity
              matmuls, where T(s) = x_left+x_right was produced by the DVE
              one block earlier. All rhs slices sit at even element offsets
              (layout below) -- an odd bf16 offset halves the PE stream rate.
  - VectorE : (a) T for the NEXT block's sample (shifted tensor_add over the
              ghost-padded state), (b) x_new(s) = bf16 copy of psum(s),
              which frees the PSUM slot one block before the PE needs it.
              GpSimd is deliberately idle: concurrent GpSimd tensor ops
              degrade DVE throughput (shared SBUF port, measured).
  - ScalarE : Dirichlet mask (zero col 0 of global rows 128..383) +
              refresh of the two ghost columns of x_new.

Grid layout: [128 partitions x (4 chunks * 516)] bf16, grid row r = 128*c+p.
Chunk layout: [0: pad, 1: ghost-left (copy of col 1), 2..513: cols 0..511,
514: ghost-right (copy of col 510), 515: pad]. Data starts at an EVEN
position (DOFF=2) for the PE; the ghosts make the horizontal reflect-add a
single shifted tensor_add.
"""
import sys
import numpy as np

if "/opt/trn_rl_repo" not in sys.path:
    sys.path.insert(0, "/opt/trn_rl_repo")

from contextlib import ExitStack

import ml_dtypes
import concourse.bass as bass
import concourse.bacc as bacc
import concourse.tile as tile
import concourse.mybir as mybir
from concourse.bass_utils import run_bass_kernel_spmd

NX = 512
P = 128
NCHUNK = NX // P  # 4
PW = NX + 4       # padded chunk width
DOFF = 2          # data offset inside a chunk (even -> 4B-aligned bf16)
BATCH = 32
NCORES = 8
SPC = BATCH // NCORES  # samples per core = 4
W = NCHUNK * NX        # 2048 compact free elems
WP = NCHUNK * PW       # 2064 padded free elems
WPA = 2304             # state tile free size (512B-multiple base stride)

BF16 = mybir.dt.bfloat16
F32 = mybir.dt.float32
NP_BF16 = ml_dtypes.bfloat16


def _build_consts() -> np.ndarray:
    """[128, 768] bf16: A_top^T | A_mid^T | A_bot^T | ct | cb | 0.25*I."""
    a_mid = np.zeros((P, P), dtype=np.float32)
    for i in range(P):
        if i > 0:
            a_mid[i, i - 1] = 0.25
        if i < P - 1:
            a_mid[i, i + 1] = 0.25
    a_top = a_mid.copy()
    a_top[0, 1] = 0.5  # reflect: row 0 vertical sum = 2*x[1]
    a_bot = a_mid.copy()
    a_bot[P - 1, P - 2] = 0.5
    consts = np.zeros((P, 768), dtype=np.float32)
    consts[:, 0:128] = a_top.T
    consts[:, 128:256] = a_mid.T
    consts[:, 256:384] = a_bot.T
    # ct: [K=64 (partitions 64..127), M=128]; only partition 127 nonzero ->
    # out partition 0 += 0.25 * x[127, prev chunk].
    consts[127, 384 + 0] = 0.25
    # cb: [K=1 (partition 0), M=128] -> out partition 127 += 0.25 * x[0, next]
    consts[0, 512 + 127] = 0.25
    # Wq: 0.25 * I for the horizontal-sum accumulation (0.25 exact in bf16)
    consts[:, 640:768] = 0.25 * np.eye(P, dtype=np.float32)
    return consts.astype(NP_BF16)


def _build(n_iter: int):
    nc = bacc.Bacc("TRN2", target_bir_lowering=False, debug=False,
                   num_devices=NCORES)

    heat_d = nc.dram_tensor("heat", [SPC, NCHUNK, P, PW], BF16,
                            kind="ExternalInput")
    cst_d = nc.dram_tensor("consts", [P, 768], BF16, kind="ExternalInput")
    out_d = nc.dram_tensor("out", [SPC, NCHUNK, P, NX], BF16,
                           kind="ExternalOutput")

    with tile.TileContext(nc) as tc:
        with ExitStack() as ctx:
            state = ctx.enter_context(tc.tile_pool(name="state", bufs=1))
            tpool = ctx.enter_context(tc.tile_pool(name="tpool", bufs=3))
            ppool = ctx.enter_context(
                tc.tile_pool(name="ppool", bufs=2, space=bass.MemorySpace.PSUM))

            cst = state.tile([P, 768], BF16, tag="cst")
            nc.sync.dma_start(cst[:], cst_d.ap())
            lhs_top = cst[:, 0:128]
            lhs_mid = cst[:, 128:256]
            lhs_bot = cst[:, 256:384]
            lhs_ct = cst[64:128, 384:512]
            lhs_cb = cst[0:1, 512:640]
            lhs_q = cst[:, 640:768]

            xa, xb = [], []
            for s in range(SPC):
                x0 = state.tile([P, WPA], BF16, tag=f"xa{s}", name=f"xa{s}")
                nc.sync.dma_start(
                    x0[:, 0:WP].rearrange("p (c j) -> p c j", c=NCHUNK),
                    heat_d.ap()[s].rearrange("c p j -> p c j"))
                xa.append(x0)
                xb.append(state.tile([P, WPA], BF16, tag=f"xb{s}",
                                     name=f"xb{s}"))

            def xsl(x, c):  # chunk c's 512 data cols in the padded tile
                return x[:, c * PW + DOFF: c * PW + DOFF + NX]

            def make_T(xsrc):
                """Emit the DVE H-add producing T = x_left + x_right."""
                T = tpool.tile([P, W], BF16, tag="T", name="T")
                t3 = T.rearrange("p (c j) -> p c j", c=NCHUNK)
                x3 = xsrc[:, 0:WP].rearrange("p (c j) -> p c j", c=NCHUNK)
                nc.vector.tensor_add(
                    t3[:, :, 0:NX], x3[:, :, 1:NX + 1], x3[:, :, 3:NX + 3])
                return T

            cur, nxt = xa, xb
            prev_T = make_T(cur[0])  # bootstrap: T for block (t=0, s=0)
            for t in range(n_iter):
                for s in range(SPC):
                    x, xn = cur[s], nxt[s]

                    psum = ppool.tile([P, W], F32, tag="P", name="psum")

                    def psl(c):
                        return psum[:, c * NX:(c + 1) * NX]

                    # --- vertical: tridiagonal + corners ---
                    nc.tensor.matmul(psl(0), lhs_top, xsl(x, 0),
                                     start=True, stop=False)
                    nc.tensor.matmul(psl(1), lhs_mid, xsl(x, 1),
                                     start=True, stop=False)
                    nc.tensor.matmul(psl(2), lhs_mid, xsl(x, 2),
                                     start=True, stop=False)
                    nc.tensor.matmul(psl(3), lhs_bot, xsl(x, 3),
                                     start=True, stop=False)
                    nc.tensor.matmul(psl(1), lhs_ct, xsl(x, 0)[64:128],
                                     start=False, stop=False)
                    nc.tensor.matmul(psl(2), lhs_ct, xsl(x, 1)[64:128],
                                     start=False, stop=False)
                    nc.tensor.matmul(psl(3), lhs_ct, xsl(x, 2)[64:128],
                                     start=False, stop=False)
                    nc.tensor.matmul(psl(0), lhs_cb, xsl(x, 1)[0:1],
                                     start=False, stop=False)
                    nc.tensor.matmul(psl(1), lhs_cb, xsl(x, 2)[0:1],
                                     start=False, stop=False)
                    nc.tensor.matmul(psl(2), lhs_cb, xsl(x, 3)[0:1],
                                     start=False, stop=False)
                    # --- horizontal: psum += 0.25 * T (T from prev block) ---
                    for c in range(NCHUNK):
                        nc.tensor.matmul(psl(c), lhs_q,
                                         prev_T[:, c * NX:(c + 1) * NX],
                                         start=False, stop=True)

                    # --- DVE: T for the next block (keeps PE fed) ---
                    last_block = (t == n_iter - 1 and s == SPC - 1)
                    if not last_block:
                        nx_src = cur[s + 1] if s < SPC - 1 else nxt[0]
                        prev_T = make_T(nx_src)

                    # --- ScalarE: x_new = bf16(psum); frees the PSUM slot.
                    # On ScalarE (not DVE) so the DVE's T production plus
                    # this copy both fit under the warm-PE block time ---
                    xn3 = xn[:, 0:WP].rearrange("p (c j) -> p c j", c=NCHUNK)
                    p3 = psum.rearrange("p (c j) -> p c j", c=NCHUNK)
                    nc.scalar.copy(xn3[:, :, DOFF:NX + DOFF], p3)

                    # --- ScalarE: Dirichlet mask + ghost refresh ---
                    nc.scalar.mul(xn[:, PW + DOFF:2 * PW + DOFF + 1:PW],
                                  xn[:, PW + DOFF:2 * PW + DOFF + 1:PW], 0.0)
                    nc.scalar.copy(xn3[:, :, 1:2], xn3[:, :, 3:4])
                    nc.scalar.copy(xn3[:, :, 514:515], xn3[:, :, 512:513])
                cur, nxt = nxt, cur

            for s in range(SPC):
                nc.sync.dma_start(
                    out_d.ap()[s].rearrange("c p j -> p c j"),
                    cur[s][:, 0:WP].rearrange(
                        "p (c j) -> p c j", c=NCHUNK)[:, :, DOFF:NX + DOFF])

    nc.compile()
    return nc


_CACHE: dict = {}


def _get_nc(n_iter: int):
    if n_iter not in _CACHE:
        _CACHE[n_iter] = _build(n_iter)
    return _CACHE[n_iter]


def _prep_heat(heat: np.ndarray) -> np.ndarray:
    """[B,512,512] fp32 -> [B,4,128,516] bf16 padded, masked, with ghosts."""
    b = heat.shape[0]
    h = heat.copy()
    h[:, 128:384, 0] = 0.0  # x0 = heat * G
    hc = h.reshape(b, NCHUNK, P, NX)
    hp = np.zeros((b, NCHUNK, P, PW), dtype=np.float32)
    hp[..., DOFF:NX + DOFF] = hc
    hp[..., DOFF - 1] = hc[..., 1]         # ghost-left = col 1
    hp[..., NX + DOFF] = hc[..., NX - 2]   # ghost-right = col 510
    return hp.astype(NP_BF16)


def run(layout, heat, n_iter, trace=False):
    n_iter = int(n_iter)
    heat = np.ascontiguousarray(np.asarray(heat, dtype=np.float32)
                                .reshape(BATCH, NX, NX))
    hp = _prep_heat(heat)
    consts = _build_consts()
    nc = _get_nc(n_iter)
    in_maps = []
    for c in range(NCORES):
        sl = slice(c * SPC, (c + 1) * SPC)
        in_maps.append({"heat": hp[sl], "consts": consts})
    res = run_bass_kernel_spmd(nc, in_maps, list(range(NCORES)), trace=trace)
    out = np.concatenate(
        [res.results[c]["out"].reshape(SPC, NX, NX) for c in range(NCORES)],
        axis=0)
    return out.astype(np.float32).reshape(BATCH, 1, NX, NX), res


def kernel(layout, heat, n_iter):
    out, _ = run(layout, heat, n_iter)
    return out
